# revision 35
# baseline (speedup 1.0000x reference)
"""AttentionMemory kernel for Trainium2 (8 NeuronCores, Bass/Tile).

Reference computation (per batch b):
    affinity[n, m] = (2 * mk[:,n]@qk[:,m] - ||mk[:,n]||^2 - ||qk[:,m]||^2) / 8
    out[n, m]      = softmax over n (memory axis)

Softmax over n is invariant to per-column constants, so the -||qk_m||^2
term is dropped.  Logits are produced by an augmented f16 matmul:
    lhsT (stationary) = [0.25 * qk ; -0.125 ; -0.125]   -> [66, Mc]
    rhs  (moving)     = [mk        ; a1     ; a2     ]  -> [66, N]
    psum[m, n] = 0.25*dot(qk_m, mk_n) - 0.125*(a1+a2)_n == logits[m, n]
with a = sum_c mk[c,n]^2 split on the host into a1 (10-mantissa-bit
exact, f16-representable) + a2 (residual).  f16 runs at 1 cycle/row.

Sharding: core c handles batch c//2, query-column half c%2 (communication
free: softmax is over the full n axis which each core holds).  Each core
writes out_c[m, n] in bf16; the host upcasts, normalizes (softmax
denominator recovered on the host) and transposes to [n, m] f32.

Per 126-row query strip: PE fills PSUM as [3,3] banks (ACT) + [2] banks
(DVE).  ACT exps the first 3024 cols (PSUM->SBUF bf16, 2 ops); stores go
out on the SP HWDGE ring.  The trailing 1008 cols are exp'd by a DVE/Pool
bit-trick chain on EVERY strip (this balances ACT at ~47.5us against the
~47us DMA store floor):

    s16 = rint(128*log2e*x + 16256)  (i16; bitcast bf16 = (1+u)*2^zi)
    w16 = (s16 & 127) | 16256        (bitcast bf16 = 1+u exactly)
    hh  = B2*w + B1                  (Pool)
    gg  = hh*w                       (DVE tt, 2x mode)
    y1  = gg + B0                    (Pool/DVE alternating)
    y2  = y1 * bitcast(s16)          (DVE tt) ~= e^x, max rel ~1e-2

q2(w) = (B2*w + B1)*w + B0 is the minimax quadratic for 2^(w-1)/w, the
exact per-segment correction of the piecewise-linear bf16 bit-trick exp.
The chain's store rides the DVE HWDGE ring so neither the in-order SP
store queue nor Pool's SWDGE path is involved.
"""

import math

import numpy as np

B, CK, H, W = 4, 64, 48, 84
N = H * W            # 4032 memory pixels (softmax axis)
HALF = N // 2        # 2016 query pixels per core
M_STRIP = 126        # output-partition strip size (16 * 126 = 2016)
N_STRIPS = HALF // M_STRIP
K_AUG = CK + 2       # 66: contraction dim incl. the two -a rows

N_CHUNK = 504        # matmul moving free dim (one PSUM bank, 8 pad cols)
ACT_COLS = 6 * N_CHUNK   # 3024 cols exp'd by ACT per strip
DVE_COLS = 2 * N_CHUNK   # 1008 cols exp'd by the DVE chain per strip

_CACHE = {}

# strips whose whole quadratic (hh, gg, y1) runs on Pool instead of DVE
# (none in the last two strips: the tail needs the short DVE-local chain)
POOL_POLY_STRIPS = (1, 3, 5, 7, 9, 11, 13)
CHAIN_BUFS = 8
DV_BUFS = 8
EXP_BUFS = 4
N_WARMUP = 6
M_CHUNK_PLAN = [(0, 1), (1, 8)]
Q_HEAD = 504

LOG2E = 1.0 / math.log(2.0)
# minimax quadratic for 2^(w-1)/w on [1,2): y1 = (B2*w + B1)*w + B0
B2C = 0.2337633580
B1C = -0.6946023881
B0C = 1.4574951935


def _build_nc():
    import concourse.bacc as bacc
    import concourse.mybir as mybir
    import concourse.tile as tile

    f32 = mybir.dt.float32
    f16 = mybir.dt.float16
    bf16 = mybir.dt.bfloat16
    i16 = mybir.dt.int16
    Exp = mybir.ActivationFunctionType.Exp
    Alu = mybir.AluOpType

    nc = bacc.Bacc("TRN2", target_bir_lowering=False, debug=False)

    q_d = nc.dram_tensor("q", [K_AUG, HALF], f16, kind="ExternalInput")
    m_d = nc.dram_tensor("m", [K_AUG, N], f16, kind="ExternalInput")
    out_d = nc.dram_tensor("out_c", [HALF, N], bf16, kind="ExternalOutput")

    with tile.TileContext(nc) as tc:
        with (
            tc.tile_pool(name="singles", bufs=1) as singles,
            tc.tile_pool(name="psum", bufs=2, space="PSUM") as psum_pool,
            tc.tile_pool(name="psum2", bufs=2, space="PSUM") as psum2_pool,
            tc.tile_pool(name="exp", bufs=EXP_BUFS) as exp_pool,
            tc.tile_pool(name="chain", bufs=CHAIN_BUFS) as chain_pool,
            tc.tile_pool(name="dve_out", bufs=DV_BUFS) as dve_pool,
        ):
            # --- inputs, staged by first use.  q head rides the Pool SWDGE
            # ring so its DGE overlaps the SP ring; m arrives in 504-col
            # chunks so the first matmul only waits on 1/8 of it -------------
            q_s = singles.tile([K_AUG, HALF], f16)
            m_s = singles.tile([K_AUG, N], f16)
            # m chunk 0 leads on the SP ring (shortest DGE path) so the first
            # matmul fires ASAP; the q head rides the ACT ring whose HWDGE
            # overlaps SP's; the rest follows on SP
            nc.sync.dma_start(out=m_s[:, :N_CHUNK], in_=m_d[:, :N_CHUNK])
            nc.gpsimd.dma_start(out=q_s[:, :Q_HEAD], in_=q_d[:, :Q_HEAD])
            for c0c, c1c in M_CHUNK_PLAN[1:]:
                sl = slice(c0c * N_CHUNK, c1c * N_CHUNK)
                nc.sync.dma_start(out=m_s[:, sl], in_=m_d[:, sl])
            nc.sync.dma_start(out=q_s[:, Q_HEAD:], in_=q_d[:, Q_HEAD:])

            # --- prewarm: ACT exp table load + PE pstate ramp during the
            # input DMAs ------------------------------------------------------
            wtab = singles.tile([1, 2], f32)
            nc.vector.memset(wtab, 0.0)
            nc.scalar.activation(wtab[:, 1:2], wtab[:, 0:1], Exp)
            wsrc = singles.tile([K_AUG, 256], f16)
            nc.vector.memset(wsrc, 0.0)
            wps = psum_pool.tile([M_STRIP, 1536], f32, tag="ps")
            for _ in range(N_WARMUP):
                nc.tensor.matmul(
                    wps[:, :256],
                    wsrc[:, :M_STRIP],
                    wsrc,
                    start=True,
                    stop=True,
                )

            pending_finish = []
            for s in range(N_STRIPS):
                m0 = s * M_STRIP
                q_l = q_s[:, m0 : m0 + M_STRIP]
                last = s == N_STRIPS - 1

                exp_t = exp_pool.tile([M_STRIP, ACT_COLS], bf16, tag="exp")

                def chain_piece():
                    # two 1-bank PSUM pieces -> bit-trick exp on DVE/Pool.
                    # Separate banks (psum2 bufs=2) with per-bank s16 reads
                    # halve the PE -> s16 -> PE recycle latency of the slot
                    s16 = chain_pool.tile([M_STRIP, DVE_COLS], i16, tag="s16")
                    w16 = chain_pool.tile([M_STRIP, DVE_COLS], i16, tag="w16")
                    hh = chain_pool.tile([M_STRIP, DVE_COLS], bf16, tag="hh")
                    gg = chain_pool.tile([M_STRIP, DVE_COLS], bf16, tag="gg")
                    y1 = chain_pool.tile([M_STRIP, DVE_COLS], bf16, tag="y1")
                    dv = dve_pool.tile([M_STRIP, DVE_COLS], bf16, tag="dv")
                    for j in range(2):
                        ps2 = psum2_pool.tile([M_STRIP, 512], f32, tag="ps2")
                        sl = slice((6 + j) * N_CHUNK, (7 + j) * N_CHUNK)
                        nc.tensor.matmul(
                            ps2[:, :N_CHUNK],
                            q_l,
                            m_s[:, sl],
                            start=True,
                            stop=True,
                        )
                        nc.vector.tensor_scalar(
                            s16[:, j * N_CHUNK : (j + 1) * N_CHUNK],
                            ps2[:, :N_CHUNK],
                            128.0 * LOG2E, 16256.0, Alu.mult, Alu.add,
                        )
                    nc.vector.tensor_scalar(
                        w16, s16, 127, 16256, Alu.bitwise_and, Alu.bitwise_or
                    )
                    wb = w16.bitcast(bf16)
                    # alternate the whole quadratic between Pool and DVE:
                    # Pool-poly strips cost only 2 cross-engine hops (w16->hh,
                    # y1->y2) and keep per-strip DVE work at 2143ns so DVE,
                    # Pool, and ACT all land near the ~47us DMA store floor
                    poly = nc.gpsimd if s in POOL_POLY_STRIPS else nc.vector
                    poly.tensor_scalar(hh, wb, B2C, B1C, Alu.mult, Alu.add)
                    poly.tensor_tensor(gg, hh, wb, Alu.mult)
                    poly.tensor_scalar(y1, gg, B0C, None, Alu.add)

                    # y2 is deferred one strip: emitting it now would make the
                    # in-order DVE stream wait on Pool's y1 handoff, stalling
                    # the next strip's s16 (and with it PE's ps2 refill)
                    def finish(pm0=m0, py1=y1, ps16=s16, pdv=dv):
                        nc.vector.tensor_tensor(
                            pdv, py1, ps16.bitcast(bf16), Alu.mult
                        )
                        nc.sync.dma_start(
                            out=out_d[pm0 : pm0 + M_STRIP, ACT_COLS:], in_=pdv
                        )

                    return finish

                def act_pieces():
                    for k in range(2):
                        ps = psum_pool.tile([M_STRIP, 1536], f32, tag="ps")
                        for j in range(3):
                            cj = k * 3 + j
                            sl = slice(cj * N_CHUNK, (cj + 1) * N_CHUNK)
                            nc.tensor.matmul(
                                ps[:, j * 512 : j * 512 + N_CHUNK],
                                q_l,
                                m_s[:, sl],
                                start=True,
                                stop=True,
                            )
                            if s == 0 and k == 0:
                                # ramp: exp each bank as it lands so the first
                                # store fires right after the first matmul
                                bsl = slice(cj * N_CHUNK, (cj + 1) * N_CHUNK)
                                nc.scalar.activation(
                                    exp_t[:, bsl].rearrange("p (b c) -> p b c", b=1),
                                    ps[:, j * 512 : j * 512 + N_CHUNK].rearrange(
                                        "p (b c) -> p b c", b=1
                                    ),
                                    Exp,
                                )
                                nc.sync.dma_start(
                                    out=out_d[m0 : m0 + M_STRIP, bsl],
                                    in_=exp_t[:, bsl],
                                )
                        if not (s == 0 and k == 0):
                            esl = slice(k * 3 * N_CHUNK, (k + 1) * 3 * N_CHUNK)
                            nc.scalar.activation(
                                exp_t[:, esl].rearrange("p (b c) -> p b c", b=3),
                                ps.rearrange("p (b c) -> p b c", b=3)[:, :, :N_CHUNK],
                                Exp,
                            )
                            nc.sync.dma_start(
                                out=out_d[m0 : m0 + M_STRIP, esl], in_=exp_t[:, esl]
                            )

                # chain piece first (strips >= 1): the chain matmuls act as
                # elastic filler in the in-order PE stream while the ps pool
                # waits on ACT draining the previous strip's pieces.  Strip 0
                # keeps act-first: its chain matmuls need the LAST m chunks,
                # which would head-of-line block PE on the input DMA.
                if s == 0:
                    act_pieces()
                    pending_finish.append(chain_piece())
                else:
                    finish = chain_piece()
                    # two-strips-late y2 + store: Pool's serial poly chain
                    # (~5.1us) exceeds one strip period, so a shallower
                    # deferral would stall the in-order DVE stream
                    while len(pending_finish) >= 2:
                        pending_finish.pop(0)()
                    pending_finish.append(finish)
                    act_pieces()
            for f in pending_finish:
                f()

    nc.compile()
    return nc


def _get_nc():
    if "nc" not in _CACHE:
        _CACHE["nc"] = _build_nc()
    return _CACHE["nc"]


def _round_mant(x: np.ndarray, bits: int) -> np.ndarray:
    """Round to `bits` explicit mantissa bits (exactly f16-representable)."""
    m, e = np.frexp(x.astype(np.float64))
    scale = 2.0 ** (bits + 1)
    return np.ldexp(np.round(m * scale) / scale, e)


def kernel(mk: np.ndarray, qk: np.ndarray) -> np.ndarray:
    from concourse import bass_utils

    mk = np.asarray(mk, dtype=np.float32).reshape(B, CK, N)
    qk = np.asarray(qk, dtype=np.float32).reshape(B, CK, N)
    a = np.einsum("bcn,bcn->bn", mk.astype(np.float64), mk.astype(np.float64))
    a1 = _round_mant(a, 10)
    a2 = (a - a1).astype(np.float32)

    in_maps = []
    for core in range(8):
        b, h = divmod(core, 2)
        m_aug = np.empty((K_AUG, N), np.float16)
        m_aug[:CK] = mk[b].astype(np.float16)
        m_aug[CK] = a1[b].astype(np.float16)
        m_aug[CK + 1] = a2[b].astype(np.float16)

        q_aug = np.empty((K_AUG, HALF), np.float16)
        q_aug[:CK] = (0.25 * qk[b, :, h * HALF : (h + 1) * HALF]).astype(np.float16)
        q_aug[CK:] = -0.125

        in_maps.append({"q": q_aug, "m": m_aug})

    res = bass_utils.run_bass_kernel_spmd(
        _get_nc(), in_maps, core_ids=list(range(8))
    )
    _CACHE["last_results"] = res

    out = np.empty((B, N, N), np.float32)
    for core in range(8):
        b, h = divmod(core, 2)
        e = res.results[core]["out_c"].astype(np.float32)  # [m, n] unnormalized
        e /= e.sum(axis=1, keepdims=True)
        out[b, :, h * HALF : (h + 1) * HALF] = e.T
    return out
